# revision 6
# baseline (speedup 1.0000x reference)
"""Trainium2 Bass kernel for nn_CoCovTranspose (sum of 9 tile-shifted stride-2
transpose convolutions).

Math (verified against the jax reference in numpy):
  The op decomposes per 16-output-row strip R (64 strips per image) as 18
  PSUM-accumulated matmuls over an im2col block that holds the 27 input rows
  {8R + ur + 32*(dr-1) : ur in [0,9), dr in [0,3)} x 8 input channels on
  216 K-partitions (split 128 + 88), with a 33-slot padded column layout so
  the 9 conv-transpose shift/tap variants become constant free-dim offsets
  (and the per-tile clipping at 32-column boundaries falls out of the zero
  pad slots).  Per-tile row clipping is a compile-time weight variant used
  for strips R % 4 == 3.

v2: bf16 im2col + weights (fp32 PSUM accumulate), pad-slot memsets hoisted
out of the strip loop (pads of the 8 ring buffers are zeroed once; edge
strips only memset their invalid row spans).

Sharding: data-parallel over batch: core i computes image i (8 cores, 8 images).
"""

import numpy as np

B, CI, CO, H = 8, 8, 8, 512
NSTRIP = 64          # output row strips of 16 rows each
KTOT = 216           # K partitions: dr(3) * ur(9) * ci(8)
K1 = 128             # chunk 1 partitions (dr=0 all 72; dr=1 ur 0..6)
K2 = KTOT - K1       # chunk 2 (dr=1 ur 7..8; dr=2 all 72)
GB = 18              # column groups incl. left/right halo
TS = 33              # 32 data cols + 1 zero pad slot
NBUF = 8             # im2col ring depth
# column groups: (e = Q parity, tb = kernel col tap, dc = column tile shift)
COLG = [(0, 1, -1), (0, 1, 0), (0, 1, 1),
        (1, 2, -1), (1, 2, 0), (1, 2, 1),
        (1, 0, -1), (1, 0, 0), (1, 0, 1)]

_CACHE = {}


def _bf16(a):
    import ml_dtypes
    return np.ascontiguousarray(a.astype(ml_dtypes.bfloat16))


def _build_host_weights(weights, biases):
    """W[v][cg][K=(dr,ur,ci)][M=(py,co)]; v=1 zeroes the (ta==0, ur==8) taps
    (row clip for strips R%4==3)."""
    W = np.zeros((2, 9, KTOT, 128), np.float32)
    for v in range(2):
        for cgi, (e, tb, dc) in enumerate(COLG):
            for dr3 in range(3):
                k = dr3 * 3 + (dc + 1)
                for ur in range(9):
                    for py in range(16):
                        ta = py + 1 - 2 * ur
                        if not (0 <= ta <= 2):
                            continue
                        if v == 1 and ta == 0 and ur == 8:
                            continue
                        W[v, cgi, dr3 * 72 + ur * 8:dr3 * 72 + ur * 8 + 8,
                          py * 8:py * 8 + 8] = weights[k, :, :, ta, tb]
    Wp = W.transpose(2, 0, 1, 3).reshape(KTOT, 2 * 9 * 128)
    w1 = _bf16(Wp[:K1])
    w2 = _bf16(Wp[K1:])
    bias_p = np.ascontiguousarray(np.broadcast_to(
        np.tile(biases.sum(0).astype(np.float32), 16)[:, None], (128, 512)))
    return w1, w2, bias_p


def _ur_chunks(dr3, ua, ub):
    """Split ur range [ua, ub) of row group dr3 into (tensor_idx, plo, ua, ub)
    pieces that stay within one K chunk. K = dr3*72 + ur*8 + ci."""
    out = []
    k0, k1 = dr3 * 72 + ua * 8, dr3 * 72 + ub * 8
    if k0 < K1 < k1:
        um = (K1 - dr3 * 72) // 8
        out.append((0, k0, ua, um))
        out.append((1, 0, um, ub))
    elif k1 <= K1:
        out.append((0, k0, ua, ub))
    else:
        out.append((1, k0 - K1, ua, ub))
    return out


def _build_nc(repeat=1):
    import concourse.bacc as bacc
    import concourse.tile as tile
    from concourse import mybir

    f32 = mybir.dt.float32
    bf16 = mybir.dt.bfloat16

    # Bacc (not raw Bass): its compile() runs the wait-legalization passes
    # (move_matmul_waits_to_ldweights / generate_event_semaphores) that the
    # TRN2 walrus codegen requires (max 1 sync wait per instruction).
    nc = bacc.Bacc("TRN2", target_bir_lowering=False, debug=False)
    # x is host-pre-transposed to (row, ci, col) so that the im2col fill DMA's
    # (row, ci) pair merges into one uniform-stride AP dim (3-dim DMA limit).
    x = nc.declare_dram_parameter("x", [H, CI, H], bf16, isOutput=False)
    w1 = nc.declare_dram_parameter("w1", [K1, 2 * 9 * 128], bf16, isOutput=False)
    w2 = nc.declare_dram_parameter("w2", [K2, 2 * 9 * 128], bf16, isOutput=False)
    bias = nc.declare_dram_parameter("bias", [128, 512], f32, isOutput=False)
    y = nc.declare_dram_parameter("y", [CO, 2 * H, 2 * H], f32, isOutput=True)

    with tile.TileContext(nc) as tc:
        with (
            tc.tile_pool(name="wpool", bufs=1) as wpool,
            tc.tile_pool(name="ir1p", bufs=NBUF) as ir1p,
            tc.tile_pool(name="ir2p", bufs=NBUF) as ir2p,
            tc.tile_pool(name="pspool", bufs=8, space="PSUM") as pspool,
            tc.tile_pool(name="stpool", bufs=4) as stpool,
        ):
            wsb1 = wpool.tile([K1, 2 * 9 * 128], bf16)
            nc.sync.dma_start(wsb1[:, :], w1[:, :])
            wsb2 = wpool.tile([K2, 2 * 9 * 128], bf16)
            nc.sync.dma_start(wsb2[:, :], w2[:, :])
            bias_sb = wpool.tile([128, 512], f32)
            nc.sync.dma_start(bias_sb[:, :], bias[:, :])

            # Zero the constant regions of every ring buffer once: the pad
            # slot (t=32) and the halo column groups (gb 0, 17).  The strip
            # loop's DMA fills only ever write [*, 1:17, 0:32], so these
            # stay zero across buffer reuse.
            ring = []
            for _ in range(NBUF):
                irt1 = ir1p.tile([K1, GB, TS], bf16, tag="ir1", name="z1")
                irt2 = ir2p.tile([K2, GB, TS], bf16, tag="ir2", name="z2")
                for t in (irt1, irt2):
                    nc.gpsimd.memset(t[:, :, 32:33], 0.0)
                    nc.gpsimd.memset(t[:, 0:1, 0:32], 0.0)
                    nc.gpsimd.memset(t[:, 17:18, 0:32], 0.0)
                ring.append((irt1, irt2))

            for it, R in enumerate(
                    [r for _ in range(repeat) for r in range(NSTRIP)]):
                irt = ring[it % NBUF]

                # edge strips (image top/bottom): some of the 27 rows are out
                # of range — zero the invalid row spans, then DMA the valid
                # rows.
                spans = []
                for dr3 in range(3):
                    base = 8 * R + 32 * (dr3 - 1)
                    ur_lo = min(9, max(0, -base))
                    ur_hi = max(ur_lo, min(9, H - base))
                    spans.append((base, ur_lo, ur_hi))
                # Engine APs need 32-aligned partition bases, so round the
                # invalid spans out to quadrant boundaries; the fills below
                # run after and rewrite any valid rows this over-zeroes.
                zsp = {0: [], 1: []}
                for dr3 in range(3):
                    base, ur_lo, ur_hi = spans[dr3]
                    for za, zb in ((0, ur_lo), (ur_hi, 9)):
                        if za < zb:
                            for ti, plo, ua, ub in _ur_chunks(dr3, za, zb):
                                zsp[ti].append((plo, plo + (ub - ua) * 8))
                for ti, sp in zsp.items():
                    if sp:
                        kmax = (K1, K2)[ti]
                        plo = (min(a for a, _ in sp) // 32) * 32
                        phi = min(kmax, -(-max(b for _, b in sp) // 32) * 32)
                        nc.gpsimd.memset(irt[ti][plo:phi, 1:17, 0:32], 0.0)

                for dr3 in range(3):
                    base, ur_lo, ur_hi = spans[dr3]
                    if ur_lo < ur_hi:
                        for ti, plo, ua, ub in _ur_chunks(dr3, ur_lo, ur_hi):
                            src = x[base + ua:base + ub, :, :].rearrange(
                                "u c (g t) -> (u c) g t", t=32)
                            nc.sync.dma_start(
                                irt[ti][plo:plo + (ub - ua) * 8, 1:17, 0:32], src)

                pe = pspool.tile([128, 512], f32, tag="ps")
                po = pspool.tile([128, 512], f32, tag="ps")
                v = 1 if R % 4 == 3 else 0
                nmm = {0: 0, 1: 0}
                for cgi, (e, tb, dc) in enumerate(COLG):
                    tbofs = 1 if tb == 0 else 0
                    tot = 6 if e == 0 else 12
                    dst = pe if e == 0 else po
                    for ti in range(2):
                        lhsT = (wsb1, wsb2)[ti][:, (v * 9 + cgi) * 128:
                                                (v * 9 + cgi) * 128 + 128]
                        rhs = irt[ti][:, 1 + dc:17 + dc, tbofs:tbofs + 32]
                        nc.tensor.matmul(
                            dst[:, :], lhsT, rhs,
                            start=(nmm[e] == 0), stop=(nmm[e] == tot - 1))
                        nmm[e] += 1

                st = stpool.tile([128, 512, 2], f32, tag="st")
                # bias-add + PSUM->SBUF interleave staging, split across the
                # Activation and DVE engines so neither serializes the PE.
                nc.scalar.add(st[:, :, 0], pe[:, :], bias_sb[:, 0:1])
                nc.vector.tensor_add(st[:, :, 1], po[:, :], bias_sb[:, :])
                nc.scalar.dma_start(
                    y[:, 16 * R:16 * R + 16, :].rearrange("c p q -> p c q"),
                    st.rearrange("p a b -> p (a b)"))
    nc.compile()
    return nc


def make_in_map(inp, i, w1, w2, bias_p):
    return {"x": _bf16(np.asarray(inp[i]).transpose(1, 0, 2)),
            "w1": w1, "w2": w2, "bias": bias_p}


def kernel(inp, weights, biases):
    from concourse.bass_utils import run_bass_kernel_spmd

    inp = np.ascontiguousarray(np.asarray(inp, dtype=np.float32))
    weights = np.asarray(weights, dtype=np.float32)
    biases = np.asarray(biases, dtype=np.float32)
    w1, w2, bias_p = _build_host_weights(weights, biases)

    if "nc" not in _CACHE:
        _CACHE["nc"] = _build_nc(repeat=int(
            __import__("os").environ.get("KERNEL_REPEAT", "1")))
    nc = _CACHE["nc"]

    in_maps = [make_in_map(inp, i, w1, w2, bias_p) for i in range(B)]
    res = run_bass_kernel_spmd(nc, in_maps, list(range(B)))
    out = np.stack([r["y"] for r in res.results]).astype(np.float32)
    return out


if __name__ == "__main__":
    rng = np.random.default_rng(0)
    inp = rng.standard_normal((B, CI, H, H), dtype=np.float32)
    w = (rng.standard_normal((9, CI, CO, 3, 3)) * 0.05).astype(np.float32)
    b = (rng.standard_normal((9, CO)) * 0.05).astype(np.float32)
    out = kernel(inp=inp, weights=w, biases=b)
    print(out.shape, out.dtype)


# revision 9
# speedup vs baseline: 1.4458x; 1.4458x over previous
"""Trainium2 Bass kernel for nn_CoCovTranspose (sum of 9 tile-shifted stride-2
transpose convolutions).

Math (verified against the jax reference in numpy):
  The op decomposes per 16-output-row strip R (64 strips per image) as 18
  PSUM-accumulated matmuls over an im2col block that holds the 27 input rows
  {8R + ur + 32*(dr-1) : ur in [0,9), dr in [0,3)} x 8 input channels on
  216 K-partitions (split 128 + 88), with a 33-slot padded column layout so
  the 9 conv-transpose shift/tap variants become constant free-dim offsets
  (and the per-tile clipping at 32-column boundaries falls out of the zero
  pad slots).  Per-tile row clipping is a compile-time weight variant used
  for strips R % 4 == 3.

v2: bf16 im2col + weights (fp32 PSUM accumulate), pad-slot memsets hoisted
out of the strip loop (pads of the 8 ring buffers are zeroed once; edge
strips only memset their invalid row spans).

Sharding: data-parallel over batch: core i computes image i (8 cores, 8 images).
"""

import numpy as np

B, CI, CO, H = 8, 8, 8, 512
NSTRIP = 64          # output row strips of 16 rows each
KTOT = 216           # K partitions: dr(3) * ur(9) * ci(8)
K1 = 128             # chunk 1 partitions (dr=0 all 72; dr=1 ur 0..6)
K2 = KTOT - K1       # chunk 2 (dr=1 ur 7..8; dr=2 all 72)
GB = 18              # column groups incl. left/right halo
TS = 33              # 32 data cols + 1 zero pad slot
NBUF = 8             # im2col ring depth
# column groups: (e = Q parity, tb = kernel col tap, dc = column tile shift)
COLG = [(0, 1, -1), (0, 1, 0), (0, 1, 1),
        (1, 2, -1), (1, 2, 0), (1, 2, 1),
        (1, 0, -1), (1, 0, 0), (1, 0, 1)]

_CACHE = {}


def _bf16(a):
    import ml_dtypes
    return np.ascontiguousarray(a.astype(ml_dtypes.bfloat16))


def _build_host_weights(weights, biases):
    """W[v][cg][K=(dr,ur,ci)][M=(py,co)]; v=1 zeroes the (ta==0, ur==8) taps
    (row clip for strips R%4==3)."""
    W = np.zeros((2, 9, KTOT, 128), np.float32)
    for v in range(2):
        for cgi, (e, tb, dc) in enumerate(COLG):
            for dr3 in range(3):
                k = dr3 * 3 + (dc + 1)
                for ur in range(9):
                    for py in range(16):
                        ta = py + 1 - 2 * ur
                        if not (0 <= ta <= 2):
                            continue
                        if v == 1 and ta == 0 and ur == 8:
                            continue
                        W[v, cgi, dr3 * 72 + ur * 8:dr3 * 72 + ur * 8 + 8,
                          py * 8:py * 8 + 8] = weights[k, :, :, ta, tb]
    Wp = W.transpose(2, 0, 1, 3).reshape(KTOT, 2 * 9 * 128)
    w1 = _bf16(Wp[:K1])
    w2 = _bf16(Wp[K1:])
    bias_p = np.ascontiguousarray(np.broadcast_to(
        np.tile(biases.sum(0).astype(np.float32), 16)[:, None], (128, 512)))
    return w1, w2, bias_p


def _ur_chunks(dr3, ua, ub):
    """Split ur range [ua, ub) of row group dr3 into (tensor_idx, plo, ua, ub)
    pieces that stay within one K chunk. K = dr3*72 + ur*8 + ci."""
    out = []
    k0, k1 = dr3 * 72 + ua * 8, dr3 * 72 + ub * 8
    if k0 < K1 < k1:
        um = (K1 - dr3 * 72) // 8
        out.append((0, k0, ua, um))
        out.append((1, 0, um, ub))
    elif k1 <= K1:
        out.append((0, k0, ua, ub))
    else:
        out.append((1, k0 - K1, ua, ub))
    return out


def _build_nc(repeat=1):
    import concourse.bacc as bacc
    import concourse.tile as tile
    from concourse import mybir

    f32 = mybir.dt.float32
    bf16 = mybir.dt.bfloat16

    # Bacc (not raw Bass): its compile() runs the wait-legalization passes
    # (move_matmul_waits_to_ldweights / generate_event_semaphores) that the
    # TRN2 walrus codegen requires (max 1 sync wait per instruction).
    nc = bacc.Bacc("TRN2", target_bir_lowering=False, debug=False)
    # x is host-pre-transposed to (row, ci, col) so that the im2col fill DMA's
    # (row, ci) pair merges into one uniform-stride AP dim (3-dim DMA limit).
    x = nc.declare_dram_parameter("x", [H, CI, H], bf16, isOutput=False)
    w1 = nc.declare_dram_parameter("w1", [K1, 2 * 9 * 128], bf16, isOutput=False)
    w2 = nc.declare_dram_parameter("w2", [K2, 2 * 9 * 128], bf16, isOutput=False)
    bias = nc.declare_dram_parameter("bias", [128, 512], f32, isOutput=False)
    y = nc.declare_dram_parameter("y", [CO, 2 * H, 2 * H], f32, isOutput=True)

    with tile.TileContext(nc) as tc:
        with (
            tc.tile_pool(name="wpool", bufs=1) as wpool,
            tc.tile_pool(name="ir1p", bufs=NBUF) as ir1p,
            tc.tile_pool(name="ir2p", bufs=NBUF) as ir2p,
            tc.tile_pool(name="pspool", bufs=8, space="PSUM") as pspool,
            tc.tile_pool(name="stpool", bufs=4) as stpool,
        ):
            wsb1 = wpool.tile([K1, 2 * 9 * 128], bf16)
            nc.sync.dma_start(wsb1[:, :], w1[:, :])
            wsb2 = wpool.tile([K2, 2 * 9 * 128], bf16)
            nc.sync.dma_start(wsb2[:, :], w2[:, :])
            bias_sb = wpool.tile([128, 512], f32)
            nc.sync.dma_start(bias_sb[:, :], bias[:, :])

            # Zero the constant regions of every ring buffer once: the pad
            # slot (t=32) and the halo column groups (gb 0, 17).  The strip
            # loop's DMA fills only ever write [*, 1:17, 0:32], so these
            # stay zero across buffer reuse.
            ring = []
            for _ in range(NBUF):
                irt1 = ir1p.tile([K1, GB, TS], bf16, tag="ir1", name="z1")
                irt2 = ir2p.tile([K2, GB, TS], bf16, tag="ir2", name="z2")
                for t in (irt1, irt2):
                    nc.gpsimd.memset(t[:, :, 32:33], 0.0)
                    nc.gpsimd.memset(t[:, 0:1, 0:32], 0.0)
                    nc.gpsimd.memset(t[:, 17:18, 0:32], 0.0)
                ring.append((irt1, irt2))

            for it, R in enumerate(
                    [r for _ in range(repeat) for r in range(NSTRIP)]):
                irt = ring[it % NBUF]

                # edge strips (image top/bottom): some of the 27 rows are out
                # of range — zero the invalid row spans, then DMA the valid
                # rows.
                spans = []
                for dr3 in range(3):
                    base = 8 * R + 32 * (dr3 - 1)
                    ur_lo = min(9, max(0, -base))
                    ur_hi = max(ur_lo, min(9, H - base))
                    spans.append((base, ur_lo, ur_hi))
                # Engine APs need 32-aligned partition bases, so round the
                # invalid spans out to quadrant boundaries; the fills below
                # run after and rewrite any valid rows this over-zeroes.
                zsp = {0: [], 1: []}
                for dr3 in range(3):
                    base, ur_lo, ur_hi = spans[dr3]
                    for za, zb in ((0, ur_lo), (ur_hi, 9)):
                        if za < zb:
                            for ti, plo, ua, ub in _ur_chunks(dr3, za, zb):
                                zsp[ti].append((plo, plo + (ub - ua) * 8))
                for ti, sp in zsp.items():
                    if sp:
                        kmax = (K1, K2)[ti]
                        plo = (min(a for a, _ in sp) // 32) * 32
                        phi = min(kmax, -(-max(b for _, b in sp) // 32) * 32)
                        nc.gpsimd.memset(irt[ti][plo:phi, 1:17, 0:32], 0.0)

                for dr3 in range(3):
                    base, ur_lo, ur_hi = spans[dr3]
                    if ur_lo < ur_hi:
                        for ti, plo, ua, ub in _ur_chunks(dr3, ur_lo, ur_hi):
                            src = x[base + ua:base + ub, :, :].rearrange(
                                "u c (g t) -> (u c) g t", t=32)
                            nc.sync.dma_start(
                                irt[ti][plo:plo + (ub - ua) * 8, 1:17, 0:32], src)

                pe = pspool.tile([128, 512], f32, tag="ps")
                po = pspool.tile([128, 512], f32, tag="ps")
                v = 1 if R % 4 == 3 else 0
                nmm = {0: 0, 1: 0}
                for cgi, (e, tb, dc) in enumerate(COLG):
                    tbofs = 1 if tb == 0 else 0
                    tot = 6 if e == 0 else 12
                    dst = pe if e == 0 else po
                    for ti in range(2):
                        lhsT = (wsb1, wsb2)[ti][:, (v * 9 + cgi) * 128:
                                                (v * 9 + cgi) * 128 + 128]
                        rhs = irt[ti][:, 1 + dc:17 + dc, tbofs:tbofs + 32]
                        nc.tensor.matmul(
                            dst[:, :], lhsT, rhs,
                            start=(nmm[e] == 0), stop=(nmm[e] == tot - 1))
                        nmm[e] += 1

                st = stpool.tile([128, 512, 2], f32, tag="st")
                # tensor_add (tensor_tensor) stays in DVE 1-port mode, so it
                # never starves the SWDGE descriptor generator on GpSimd.
                nc.vector.tensor_add(st[:, :, 0], pe[:, :], bias_sb[:, :])
                nc.vector.tensor_add(st[:, :, 1], po[:, :], bias_sb[:, :])
                nc.scalar.dma_start(
                    y[:, 16 * R:16 * R + 16, :].rearrange("c p q -> p c q"),
                    st.rearrange("p a b -> p (a b)"))
    nc.compile()
    return nc


def make_in_map(inp, i, w1, w2, bias_p):
    return {"x": _bf16(np.asarray(inp[i]).transpose(1, 0, 2)),
            "w1": w1, "w2": w2, "bias": bias_p}


def kernel(inp, weights, biases):
    from concourse.bass_utils import run_bass_kernel_spmd

    inp = np.ascontiguousarray(np.asarray(inp, dtype=np.float32))
    weights = np.asarray(weights, dtype=np.float32)
    biases = np.asarray(biases, dtype=np.float32)
    w1, w2, bias_p = _build_host_weights(weights, biases)

    if "nc" not in _CACHE:
        _CACHE["nc"] = _build_nc(repeat=int(
            __import__("os").environ.get("KERNEL_REPEAT", "1")))
    nc = _CACHE["nc"]

    in_maps = [make_in_map(inp, i, w1, w2, bias_p) for i in range(B)]
    res = run_bass_kernel_spmd(nc, in_maps, list(range(B)))
    out = np.stack([r["y"] for r in res.results]).astype(np.float32)
    return out


if __name__ == "__main__":
    rng = np.random.default_rng(0)
    inp = rng.standard_normal((B, CI, H, H), dtype=np.float32)
    w = (rng.standard_normal((9, CI, CO, 3, 3)) * 0.05).astype(np.float32)
    b = (rng.standard_normal((9, CO)) * 0.05).astype(np.float32)
    out = kernel(inp=inp, weights=w, biases=b)
    print(out.shape, out.dtype)


# revision 12
# speedup vs baseline: 1.5779x; 1.0914x over previous
"""Trainium2 Bass kernel for nn_CoCovTranspose (sum of 9 tile-shifted stride-2
transpose convolutions).

Math (verified against the jax reference in numpy):
  The op decomposes per 16-output-row strip R (64 strips per image) as 18
  PSUM-accumulated matmuls over an im2col block that holds the 27 input rows
  {8R + ur + 32*(dr-1) : ur in [0,9), dr in [0,3)} x 8 input channels on
  216 K-partitions (split 128 + 88), with a 33-slot padded column layout so
  the 9 conv-transpose shift/tap variants become constant free-dim offsets
  (and the per-tile clipping at 32-column boundaries falls out of the zero
  pad slots).  Per-tile row clipping is a compile-time weight variant used
  for strips R % 4 == 3.

v2: bf16 im2col + weights (fp32 PSUM accumulate), pad-slot memsets hoisted
out of the strip loop (pads of the 8 ring buffers are zeroed once; edge
strips only memset their invalid row spans).

Sharding: data-parallel over batch: core i computes image i (8 cores, 8 images).
"""

import numpy as np

B, CI, CO, H = 8, 8, 8, 512
NSTRIP = 64          # output row strips of 16 rows each
KTOT = 216           # K partitions: dr(3) * ur(9) * ci(8)
K1 = 128             # chunk 1 partitions (dr=0 all 72; dr=1 ur 0..6)
K2 = KTOT - K1       # chunk 2 (dr=1 ur 7..8; dr=2 all 72)
GB = 18              # column groups incl. left/right halo
TS = 33              # 32 data cols + 1 zero pad slot
NBUF = 8             # im2col ring depth
# column groups: (e = Q parity, tb = kernel col tap, dc = column tile shift)
COLG = [(0, 1, -1), (0, 1, 0), (0, 1, 1),
        (1, 2, -1), (1, 2, 0), (1, 2, 1),
        (1, 0, -1), (1, 0, 0), (1, 0, 1)]

_CACHE = {}


def _bf16(a):
    import ml_dtypes
    return np.ascontiguousarray(a.astype(ml_dtypes.bfloat16))


def _build_host_weights(weights, biases):
    """W[v][cg][K=(dr,ur,ci)][M=(py,co)]; v=1 zeroes the (ta==0, ur==8) taps
    (row clip for strips R%4==3)."""
    W = np.zeros((2, 9, KTOT, 128), np.float32)
    for v in range(2):
        for cgi, (e, tb, dc) in enumerate(COLG):
            for dr3 in range(3):
                k = dr3 * 3 + (dc + 1)
                for ur in range(9):
                    for py in range(16):
                        ta = py + 1 - 2 * ur
                        if not (0 <= ta <= 2):
                            continue
                        if v == 1 and ta == 0 and ur == 8:
                            continue
                        W[v, cgi, dr3 * 72 + ur * 8:dr3 * 72 + ur * 8 + 8,
                          py * 8:py * 8 + 8] = weights[k, :, :, ta, tb]
    Wp = W.transpose(2, 0, 1, 3).reshape(KTOT, 2 * 9 * 128)
    w1 = _bf16(Wp[:K1])
    # chunk 2 padded to a full 128-K tile with zero weight rows: every
    # ldweights is then a uniform [128, 128] load (matmul cost is N-only,
    # so the pad rows are free; their rhs rows are zeroed once at init).
    w2p = np.zeros((K1, 2 * 9 * 128), np.float32)
    w2p[:K2] = Wp[K1:]
    w2 = _bf16(w2p)
    bias_p = np.ascontiguousarray(np.broadcast_to(
        np.tile(biases.sum(0).astype(np.float32), 16)[:, None], (128, 512)))
    return w1, w2, bias_p


def _ur_chunks(dr3, ua, ub):
    """Split ur range [ua, ub) of row group dr3 into (tensor_idx, plo, ua, ub)
    pieces that stay within one K chunk. K = dr3*72 + ur*8 + ci."""
    out = []
    k0, k1 = dr3 * 72 + ua * 8, dr3 * 72 + ub * 8
    if k0 < K1 < k1:
        um = (K1 - dr3 * 72) // 8
        out.append((0, k0, ua, um))
        out.append((1, 0, um, ub))
    elif k1 <= K1:
        out.append((0, k0, ua, ub))
    else:
        out.append((1, k0 - K1, ua, ub))
    return out


def _enable_ldw_opt():
    """Flip walrus's --enable-ldw-opt to true (experiment: backgrounded
    weight loads).  Known to crash codegen with non-uniform K loads."""
    import concourse.bass_utils as _bu
    if getattr(_bu, "_ldw_opt_patched", False):
        return
    _orig = _bu.run_command

    def _patched(cmd, *a, **k):
        if isinstance(cmd, list):
            cmd = ["--enable-ldw-opt=true" if c == "--enable-ldw-opt=false"
                   else c for c in cmd]
        return _orig(cmd, *a, **k)

    _bu.run_command = _patched
    _bu._ldw_opt_patched = True


def _build_nc(repeat=1):
    import concourse.bacc as bacc
    import concourse.tile as tile
    from concourse import mybir

    if __import__("os").environ.get("LDW_OPT", "0") == "1":
        _enable_ldw_opt()

    f32 = mybir.dt.float32
    bf16 = mybir.dt.bfloat16

    # Bacc (not raw Bass): its compile() runs the wait-legalization passes
    # (move_matmul_waits_to_ldweights / generate_event_semaphores) that the
    # TRN2 walrus codegen requires (max 1 sync wait per instruction).
    nc = bacc.Bacc("TRN2", target_bir_lowering=False, debug=False)
    # x is host-pre-transposed to (row, ci, col) so that the im2col fill DMA's
    # (row, ci) pair merges into one uniform-stride AP dim (3-dim DMA limit).
    x = nc.declare_dram_parameter("x", [H, CI, H], bf16, isOutput=False)
    w1 = nc.declare_dram_parameter("w1", [K1, 2 * 9 * 128], bf16, isOutput=False)
    w2 = nc.declare_dram_parameter("w2", [K1, 2 * 9 * 128], bf16, isOutput=False)
    bias = nc.declare_dram_parameter("bias", [128, 512], f32, isOutput=False)
    y = nc.declare_dram_parameter("y", [CO, 2 * H, 2 * H], f32, isOutput=True)

    with tile.TileContext(nc) as tc:
        with (
            tc.tile_pool(name="wpool", bufs=1) as wpool,
            tc.tile_pool(name="ir1p", bufs=NBUF) as ir1p,
            tc.tile_pool(name="ir2p", bufs=NBUF) as ir2p,
            tc.tile_pool(name="pspool", bufs=8, space="PSUM") as pspool,
            tc.tile_pool(name="stpool", bufs=4) as stpool,
        ):
            wsb1 = wpool.tile([K1, 2 * 9 * 128], bf16)
            nc.sync.dma_start(wsb1[:, :], w1[:, :])
            wsb2 = wpool.tile([K1, 2 * 9 * 128], bf16)
            nc.sync.dma_start(wsb2[:, :], w2[:, :])
            bias_sb = wpool.tile([128, 512], f32)
            nc.sync.dma_start(bias_sb[:, :], bias[:, :])

            # Zero the constant regions of every ring buffer once: the pad
            # slot (t=32) and the halo column groups (gb 0, 17).  The strip
            # loop's DMA fills only ever write [*, 1:17, 0:32], so these
            # stay zero across buffer reuse.
            ring = []
            for _ in range(NBUF):
                irt1 = ir1p.tile([K1, GB, TS], bf16, tag="ir1", name="z1")
                irt2 = ir2p.tile([K1, GB, TS], bf16, tag="ir2", name="z2")
                for t in (irt1, irt2):
                    nc.gpsimd.memset(t[:, :, :], 0.0)
                ring.append((irt1, irt2))

            for it, R in enumerate(
                    [r for _ in range(repeat) for r in range(NSTRIP)]):
                irt = ring[it % NBUF]

                # edge strips (image top/bottom): some of the 27 rows are out
                # of range — zero the invalid row spans, then DMA the valid
                # rows.
                spans = []
                for dr3 in range(3):
                    base = 8 * R + 32 * (dr3 - 1)
                    ur_lo = min(9, max(0, -base))
                    ur_hi = max(ur_lo, min(9, H - base))
                    spans.append((base, ur_lo, ur_hi))
                # Engine APs need 32-aligned partition bases, so round the
                # invalid spans out to quadrant boundaries; the fills below
                # run after and rewrite any valid rows this over-zeroes.
                zsp = {0: [], 1: []}
                for dr3 in range(3):
                    base, ur_lo, ur_hi = spans[dr3]
                    for za, zb in ((0, ur_lo), (ur_hi, 9)):
                        if za < zb:
                            for ti, plo, ua, ub in _ur_chunks(dr3, za, zb):
                                zsp[ti].append((plo, plo + (ub - ua) * 8))
                for ti, sp in zsp.items():
                    if sp:
                        kmax = K1
                        plo = (min(a for a, _ in sp) // 32) * 32
                        phi = min(kmax, -(-max(b for _, b in sp) // 32) * 32)
                        nc.gpsimd.memset(irt[ti][plo:phi, 1:17, 0:32], 0.0)

                for dr3 in range(3):
                    base, ur_lo, ur_hi = spans[dr3]
                    if ur_lo < ur_hi:
                        for ti, plo, ua, ub in _ur_chunks(dr3, ur_lo, ur_hi):
                            src = x[base + ua:base + ub, :, :].rearrange(
                                "u c (g t) -> (u c) g t", t=32)
                            nc.sync.dma_start(
                                irt[ti][plo:plo + (ub - ua) * 8, 1:17, 0:32], src)

                pe = pspool.tile([128, 512], f32, tag="ps")
                po = pspool.tile([128, 512], f32, tag="ps")
                v = 1 if R % 4 == 3 else 0
                nmm = {0: 0, 1: 0}
                for cgi, (e, tb, dc) in enumerate(COLG):
                    tbofs = 1 if tb == 0 else 0
                    tot = 6 if e == 0 else 12
                    dst = pe if e == 0 else po
                    for ti in range(2):
                        lhsT = (wsb1, wsb2)[ti][:, (v * 9 + cgi) * 128:
                                                (v * 9 + cgi) * 128 + 128]
                        rhs = irt[ti][:, 1 + dc:17 + dc, tbofs:tbofs + 32]
                        nc.tensor.matmul(
                            dst[:, :], lhsT, rhs,
                            start=(nmm[e] == 0), stop=(nmm[e] == tot - 1))
                        nmm[e] += 1

                st = stpool.tile([128, 512, 2], f32, tag="st")
                # tensor_add (tensor_tensor) stays in DVE 1-port mode, so it
                # never starves the SWDGE descriptor generator on GpSimd.
                nc.vector.tensor_add(st[:, :, 0], pe[:, :], bias_sb[:, :])
                nc.vector.tensor_add(st[:, :, 1], po[:, :], bias_sb[:, :])
                nc.scalar.dma_start(
                    y[:, 16 * R:16 * R + 16, :].rearrange("c p q -> p c q"),
                    st.rearrange("p a b -> p (a b)"))
    nc.compile()
    return nc


def make_in_map(inp, i, w1, w2, bias_p):
    return {"x": _bf16(np.asarray(inp[i]).transpose(1, 0, 2)),
            "w1": w1, "w2": w2, "bias": bias_p}


def kernel(inp, weights, biases):
    from concourse.bass_utils import run_bass_kernel_spmd

    inp = np.ascontiguousarray(np.asarray(inp, dtype=np.float32))
    weights = np.asarray(weights, dtype=np.float32)
    biases = np.asarray(biases, dtype=np.float32)
    w1, w2, bias_p = _build_host_weights(weights, biases)

    if "nc" not in _CACHE:
        _CACHE["nc"] = _build_nc(repeat=int(
            __import__("os").environ.get("KERNEL_REPEAT", "1")))
    nc = _CACHE["nc"]

    in_maps = [make_in_map(inp, i, w1, w2, bias_p) for i in range(B)]
    res = run_bass_kernel_spmd(nc, in_maps, list(range(B)))
    out = np.stack([r["y"] for r in res.results]).astype(np.float32)
    return out


if __name__ == "__main__":
    rng = np.random.default_rng(0)
    inp = rng.standard_normal((B, CI, H, H), dtype=np.float32)
    w = (rng.standard_normal((9, CI, CO, 3, 3)) * 0.05).astype(np.float32)
    b = (rng.standard_normal((9, CO)) * 0.05).astype(np.float32)
    out = kernel(inp=inp, weights=w, biases=b)
    print(out.shape, out.dtype)
